# revision 45
# baseline (speedup 1.0000x reference)
"""Trainium2 Bass kernel for the per-sample dynamic-depthwise-conv block.

Computation (per sample b):
    att  = sigmoid(lrelu(v @ ca_w1.T) @ ca_w2.T)            # [b, 64]
    kern = (lrelu(v @ k_w1.T) @ k_w2.T).reshape(b*64,1,3,3) # per-(b,c) 3x3
    y    = lrelu(depthwise3x3(x0 * att, kern))
    out  = conv1x1(y, conv_w) + conv_b

Strategy: data-parallel over batch across 8 cores (4 samples/core).  On each
core, samples are processed in 2 "pairs"; a pair's 2x64 channels fill the 128
SBUF partitions.  The attention gate is folded into the generated tap weights
(dw(att*x) == att*dw(x) per channel).  x is pre-padded on the host to width
130 (zero side columns) so each input DMA moves full 4160-byte contiguous
row blocks; the vertical one-row borders are zeroed in SBUF.

The 9 depthwise taps are spread over four engines plus the DMA engines so no
single engine owns more than ~70% of the span:
  - PE: the 4 odd-column taps (0,1),(1,1),(2,1),(1,2) as PSUM-accumulated
    matmuls with diagonal bf16 lhsT against shifted views of the padded
    tile, plus one identity-matmul injection of the SBUF partial per 8-row
    group, plus the final 1x1 conv.
  - DVE: the 4 even-column taps (0,0),(0,2),(1,0),(2,2) as 4x-mode
    tensor_scalar products, merged with two 2x-mode adds.
  - GpSimd(Pool): tap (2,0) fused product+merge into the first DVE product
    via scalar_tensor_tensor.
  - DMA (gpsimd SWDGE): one in-place accumulate merge (P2 += P3) rides the
    otherwise ~60%-idle DMA engines.
Leaky-relu runs on the Scalar engine (Prelu, PSUM->SBUF, bf16) over
[128,1024] groups; the final 1x1 conv is one matmul per 8-row group with a
block-diagonal [conv_w.T, conv_w.T] lhsT; conv bias rides the Scalar
engine's Identity activation during the PSUM->SBUF copy; outputs leave as
bf16 and are widened to fp32 on the host.

Queue layout: x row-block loads on the SP HWDGE queue (emitted first so
nothing delays them), constants + output stores on the Act HWDGE queue,
gather DMAs on SP after the loads, the merge-accumulate on the Pool SWDGE
queue.  The tiny MLP runs as one fused [64,72] matmul (k_w1|ca_w1) + Prelu,
then sigmoid/kern stages; per-pair tap-scalar prep is software-pipelined one
iteration ahead (ping-pong A/B buffer sets) so the MLP->gather->diag chain
never sits on the PE critical path at the loop seam.
"""

import sys

if "/opt/trn_rl_repo" not in sys.path:
    sys.path.append("/opt/trn_rl_repo")

import numpy as np
import ml_dtypes

B, C, H, W = 32, 64, 128, 128
KK = 3
RED = 8
N_CORES = 8
BPC = B // N_CORES          # samples per core (4)
PAIRS = BPC // 2            # sample pairs per core (2)
HP, WP = H + 2, W + 2       # padded image dims (130); width padded on host
RPG = 8                     # output rows per PE group -> N = 8*128 = 1024
NGRP = H // RPG             # 16 groups per pair
BLK = 32                    # DVE partial block rows
NBLK = H // BLK             # 4 blocks per pair

# tap assignment (di, dj); t = di*3 + dj
PE_TAPS = [(0, 1), (1, 1), (2, 1), (1, 2)]    # odd dj -> PE diag matmuls
DVE_TAPS = [(0, 0), (0, 2), (1, 0), (2, 2)]   # even dj -> DVE 4x muls
POOL_TAP = (2, 0)                              # fused mul+add on gpsimd
import os as _os
DMA_MERGE = _os.environ.get("K_DMA_MERGE", "0") == "1"   # SWDGE accum-DMA merge: compiles + small-probe passes, but the full kernel NEFF dies at runtime with it on
GATHER_MERGED = _os.environ.get("K_GATHER_MERGED", "1") == "1"
GATHER_POOL = _os.environ.get("K_GATHER_POOL", "1") == "1"
PAIR1_POOL = _os.environ.get("K_PAIR1_POOL", "1") == "1"
XSPLIT = 8                                     # x row-block DMAs per pair

_CACHE = {}


class _Env:
    pass


def _build(repeat=1, n_taps=9, unroll=None):
    import concourse.bass as bass  # noqa: F401
    from concourse import bacc, tile, mybir

    f32 = mybir.dt.float32
    bf16 = mybir.dt.bfloat16

    e = _Env()
    e.mybir = mybir
    e.AF = mybir.ActivationFunctionType
    e.ALU = mybir.AluOpType
    e.f32 = f32
    e.bf16 = bf16

    nc = bacc.Bacc(None, target_bir_lowering=False, debug=False)
    e.nc = nc

    e.x = nc.dram_tensor("x", [BPC, C, H, WP], bf16, kind="ExternalInput")
    e.vt = nc.dram_tensor("vt", [C, BPC], bf16, kind="ExternalInput")
    e.w1cat = nc.dram_tensor("w1cat", [C, C + RED], bf16,
                             kind="ExternalInput")
    e.caw2t = nc.dram_tensor("caw2t", [RED, 2 * C], bf16,
                             kind="ExternalInput")
    e.kw2t = nc.dram_tensor("kw2t", [C, C * KK * KK], bf16,
                            kind="ExternalInput")
    e.convt = nc.dram_tensor("convt", [128, 128], bf16, kind="ExternalInput")
    e.bcol = nc.dram_tensor("bcol", [128, 1], f32, kind="ExternalInput")
    e.eye = nc.dram_tensor("eye", [128, 128], f32, kind="ExternalInput")
    e.eyebf = nc.dram_tensor("eyebf", [128, 128], bf16, kind="ExternalInput")
    e.out = nc.dram_tensor("out", [BPC, C, H, W], bf16, kind="ExternalOutput")

    with tile.TileContext(nc) as tc:
        with (
            tc.tile_pool(name="consts", bufs=1) as consts,
            tc.tile_pool(name="stage", bufs=1) as stage,
            tc.tile_pool(name="diags", bufs=1) as diags,
            tc.tile_pool(name="xin", bufs=2) as xin,
            tc.tile_pool(name="parts", bufs=2) as parts,
            tc.tile_pool(name="ys", bufs=3) as ys,
            tc.tile_pool(name="os", bufs=3) as osb,
            tc.tile_pool(name="psA", bufs=2, space="PSUM") as psA,
            tc.tile_pool(name="psB", bufs=2, space="PSUM") as psB,
        ):
            e.consts, e.stage, e.diags = consts, stage, diags
            e.xin, e.parts, e.ys, e.osb = xin, parts, ys, osb
            e.psA, e.psB = psA, psB

            # ---- constants into SBUF (Act HWDGE queue; keeps SP free for
            # the x row-block loads that gate first compute) ----
            e.vt_sb = consts.tile([C, BPC], bf16)
            e.w1cat_sb = consts.tile([C, C + RED], bf16)
            e.caw2t_sb = consts.tile([C + RED, 2 * C], bf16)  # rows 64:72
            e.kw2t_sb = consts.tile([C, C * KK * KK], bf16)
            e.scratch = consts.tile([128, 512], bf16)     # PE warmup fodder
            e.convt_sb = consts.tile([128, 128], bf16)
            e.bcol_sb = consts.tile([128, 1], f32)
            e.eye_sb = consts.tile([128, 128], f32)
            e.eyebf_sb = consts.tile([128, 128], bf16)
            # ordered by when the prep chain needs them: MLP weights first,
            # then eye (diag builds), then the conv-stage constants.  On the
            # SP queue *before* the x row-block loads: the x transfers are
            # 1.5us each on the shared DMA FIFO and would starve these tiny
            # loads for ~10us if queued first.
            for t, d in (
                (e.vt_sb[:], e.vt), (e.w1cat_sb[:], e.w1cat),
                (e.caw2t_sb[C:C + RED, :], e.caw2t),
                (e.eye_sb[:], e.eye), (e.kw2t_sb[:], e.kw2t),
                (e.eyebf_sb[:], e.eyebf), (e.convt_sb[:], e.convt),
                (e.bcol_sb[:], e.bcol),
            ):
                nc.sync.dma_start(out=t, in_=d.ap())

            e.xv = e.x.ap().rearrange("(pr s) c h w -> pr (s c) h w",
                                      pr=PAIRS)
            e.ov = e.out.ap().rearrange("(pr s) c h w -> pr (s c) h w",
                                        pr=PAIRS)

            setA = _alloc_bufset(e, "A")
            setB = _alloc_bufset(e, "B")

            # PE p-state warmup: junk matmuls on an unwritten scratch tile
            # keep the PE continuously busy from t~0.5us so it reaches full
            # clock (2.4 GHz needs 3us of continuous execution) before the
            # real tap stream begins.  Interleaved through the initial prep
            # so no real instruction is delayed by more than one 512-col
            # matmul.
            nc.gpsimd.memset(e.scratch[:], 0.0)

            def warm():
                psw = e.psA.tile([128, 512], f32, tag="pa", name="wm")
                nc.tensor.matmul(psw[:], lhsT=e.scratch[:, 0:128],
                                 rhs=e.scratch[:], start=True, stop=True,
                                 skip_group_check=True)

            for _ in range(4):
                warm()
            xts0 = _load_x0(e, n_taps)
            _prep(e, setA, warm=warm)
            if repeat == 1:
                n_un = unroll or 1
                sets = [setA, setB]
                for u in range(n_un):
                    stp = (_prep_steps(e, sets[(u + 1) % 2])
                           if u + 1 < n_un else None)
                    _main(e, sets[u % 2], stp, variant=n_taps,
                          first=(u == 0), xts=(xts0 if u == 0 else None))
            else:
                assert unroll is None
                half, odd = divmod(repeat, 2)
                if half > 0:
                    with tc.For_i(0, half, 1):
                        _main(e, setA, _prep_steps(e, setB), variant=n_taps)
                        _main(e, setB, _prep_steps(e, setA), variant=n_taps)
                if odd:
                    _main(e, setA, variant=n_taps)

    nc.compile()
    return nc


def _alloc_bufset(e, nm):
    """Per-iteration tap-scalar buffers: diag tiles for the PE taps and the
    d-column tile per pair."""
    s = _Env()
    s.diag = [{} for _ in range(PAIRS)]
    s.dcols = []
    for pr in range(PAIRS):
        s.dcols.append(
            e.stage.tile([128, KK * KK], e.f32, tag=f"d{nm}{pr}",
                         name=f"d{nm}{pr}"))
        # pair0 carries diags for all 9 taps: the kernel-start block runs
        # entirely on PE (no partial-injection dependency)
        tl = PE_TAPS + ([POOL_TAP] + DVE_TAPS if pr == 0 else [])
        for (di, dj) in tl:
            t = di * KK + dj
            s.diag[pr][(di, dj)] = e.diags.tile(
                [128, 128], e.bf16, tag=f"diag{nm}{pr}_{t}",
                name=f"diag{nm}{pr}_{t}")
    return s


def _prep_steps(e, bset, first=False):
    """MLP + per-pair tap-scalar prep as a list of emit-closures so the
    caller can interleave them between main-loop groups (hides the
    matmul->activation round-trip latencies behind main-loop work).
    `first` additionally builds pair0's pool/DVE-tap diags for the all-PE
    kernel-start block."""
    nc, AF, f32 = e.nc, e.AF, e.f32
    NK = C * KK * KK
    st = _Env()
    steps = []

    def s_h():
        # fused hidden: h[0:64] = lrelu(k_w1 @ v), h[64:72] = lrelu(ca_w1 @ v)
        ps_h = e.psA.tile([C + RED, BPC], f32, tag="pa", name="ps_h")
        nc.tensor.matmul(ps_h[:], lhsT=e.w1cat_sb[:], rhs=e.vt_sb[:],
                         start=True, stop=True)
        st.h = e.stage.tile([C + RED, BPC], e.bf16, tag="h", name="h")
        nc.scalar.activation(st.h[:], ps_h[:], AF.Prelu, alpha=0.1)

    def s_att():
        # lhsT = [caw2t | caw2t] so the attention lands duplicated across
        # both 64-partition halves: att2[p, b] = att[p % 64, b].  The
        # per-pair attpp columns then come from cheap same-partition Act
        # copies instead of partition-scatter DMAs.
        ps_att = e.psA.tile([2 * C, BPC], f32, tag="pa", name="ps_att")
        nc.tensor.matmul(ps_att[:], lhsT=e.caw2t_sb[C:C + RED, :],
                         rhs=st.h[C:C + RED, :], start=True, stop=True)
        st.att = e.stage.tile([2 * C, BPC], f32, tag="att", name="att")
        nc.scalar.activation(st.att[:], ps_att[:], AF.Sigmoid)

    def s_kern():
        ps_k = e.psA.tile([BPC, NK], f32, tag="pa", name="ps_k")
        nc.tensor.matmul(ps_k[:, 0:512], lhsT=st.h[0:C, :],
                         rhs=e.kw2t_sb[:, 0:512], start=True, stop=True)
        nc.tensor.matmul(ps_k[:, 512:NK], lhsT=st.h[0:C, :],
                         rhs=e.kw2t_sb[:, 512:NK], start=True, stop=True)
        st.kern = e.stage.tile([BPC, NK], f32, tag="kern", name="kern")
        nc.scalar.activation(st.kern[:], ps_k[:], AF.Copy)

    def s_gather():
        # partition-scatter gathers ride the Pool SWDGE queue: its SEQ
        # dispatch is ~25ns vs the ~650ns HWDGE config that would stall
        # the SP/Act queues mid-prep
        st.dtap = e.stage.tile([128, PAIRS, KK * KK], f32, tag="dtap",
                               name="dtap")
        st.attpp = e.stage.tile([128, PAIRS], f32, tag="attpp",
                                name="attpp")
        dma = nc.gpsimd.dma_start if GATHER_POOL else nc.sync.dma_start
        for pr in range(PAIRS):
            if GATHER_MERGED:
                src = st.kern[2 * pr:2 * pr + 2, :].rearrange(
                    "s (c t) -> s c t", c=C)
                dma(out=st.dtap[:, pr:pr + 1, :], in_=src)
            else:
                for sdx in range(2):
                    b = pr * 2 + sdx
                    src = st.kern[b:b + 1, :].rearrange(
                        "o (c t) -> o c t", c=C)
                    dma(out=st.dtap[C * sdx:C * (sdx + 1), pr:pr + 1, :],
                        in_=src)
            for sdx in range(2):
                b = pr * 2 + sdx
                nc.scalar.activation(
                    st.attpp[C * sdx:C * (sdx + 1), pr:pr + 1],
                    st.att[C * sdx:C * (sdx + 1), b:b + 1], AF.Copy)

    def s_dcol(pr):
        def f():
            nc.vector.tensor_scalar_mul(
                bset.dcols[pr][:], st.dtap[:, pr, :],
                st.attpp[:, pr:pr + 1])
        return f

    def s_diag(pr, di, dj):
        def f():
            t = di * KK + dj
            nc.scalar.activation(
                bset.diag[pr][(di, dj)][:], e.eye_sb[:], AF.Copy,
                scale=bset.dcols[pr][:, t:t + 1])
        return f

    steps = [s_h, s_att, s_kern, s_gather]
    for pr in range(PAIRS):
        steps.append(s_dcol(pr))
        tl = list(PE_TAPS)
        if first and pr == 0:
            tl += [POOL_TAP] + DVE_TAPS
        for (di, dj) in tl:
            steps.append(s_diag(pr, di, dj))
    return steps


def _prep(e, bset, warm=None):
    for i, f in enumerate(_prep_steps(e, bset, first=True)):
        f()
        if warm is not None and i < 22:
            warm()


def _load_x0(e, variant):
    """Allocate both pairs' padded tiles, zero the borders, and start
    pair0's row-block loads on the SP HWDGE queue.  pair1's loads are
    emitted later (in _main) on the Pool SWDGE queue so their transfers
    enter the shared DMA-engine FIFO *after* the prep gathers — otherwise
    the tiny gather scatters queue behind ~24us of x traffic."""
    nc, bf16 = e.nc, e.bf16
    xts = []
    rstep = H // XSPLIT
    for pr in range(PAIRS):
        xt = e.xin.tile([128, HP, WP], bf16, tag="xt", name=f"xt{pr}")
        xts.append(xt)
        # zero the top/bottom border rows (side columns are zero from the
        # host-side width padding)
        nc.gpsimd.memset(xt[:, 0, :], 0.0)
        nc.gpsimd.memset(xt[:, HP - 1, :], 0.0)
    for k in (range(XSPLIT) if variant != 1 else []):
        r0 = k * rstep
        nc.sync.dma_start(
            out=xts[0][:, 1 + r0:1 + r0 + rstep, :],
            in_=e.xv[0, :, r0:r0 + rstep, :])
    return xts


def _main(e, bset, steps=None, variant=9, first=False, xts=None):
    """Two-pair main loop reading tap scalars from `bset`.  `steps` are
    next-iteration prep emitters, drained one per PE group."""
    nc, AF, ALU, f32, bf16 = e.nc, e.AF, e.ALU, e.f32, e.bf16
    steps = list(steps) if steps else []
    pend = []
    NW = (RPG // 2) * W

    t_pool = POOL_TAP[0] * KK + POOL_TAP[1]

    if xts is None:
        xts = _load_x0(e, variant)
    rstep = H // XSPLIT
    for pr in range(1, PAIRS):
        dma = nc.gpsimd.dma_start if PAIR1_POOL else nc.sync.dma_start
        for k in (range(XSPLIT) if variant != 1 else []):
            r0 = k * rstep
            dma(out=xts[pr][:, 1 + r0:1 + r0 + rstep, :],
                in_=e.xv[pr, :, r0:r0 + rstep, :])

    # non-PE partial per BLK-row block:
    #   P1 = x(0,0)*k00            (DVE 4x)
    #   Q  = x(2,0)*k20            (Pool tensor_scalar product)
    #   P2 = x(0,2)*k02            (DVE 4x)
    #   P3 = x(1,0)*k10            (DVE 4x)
    #   P2 += P3                   (SWDGE dma accum or DVE add)
    #   P4 = x(2,2)*k22            (DVE 4x)
    #   P2 += P4; P2 += Q; P2 += P1   (DVE adds)
    # (scalar_tensor_tensor is not in the Pool engine's ISA, so the (2,0)
    # tap is a plain product folded by DVE.)  All P1 products are emitted
    # first, then per block the remaining muls with the adds software-
    # pipelined one block behind so the in-order DVE stream never stalls on
    # the Pool/DMA merge latency.  When `first` (kernel start), pair0/block0
    # runs entirely on PE diag matmuls instead — the first injects would
    # otherwise wait on the whole DVE/Pool chain.
    part_of = {}
    kcs = bset.dcols

    def xv(pr, b, di, dj):
        r0 = b * BLK
        return xts[pr][:, r0 + di:r0 + di + BLK, dj:dj + W]

    blks = [(pr, b) for pr in range(PAIRS)
            for b in ([] if variant == 5 else range(NBLK))
            if not (first and pr == 0 and b == 0)]

    p1s = {}
    for pr, b in blks:
        (d0, j0) = DVE_TAPS[0]
        p1 = e.parts.tile([128, BLK, W], bf16, tag="p1", bufs=3,
                          name=f"p1_{pr}{b}")
        nc.vector.tensor_scalar_mul(
            p1[:], xv(pr, b, d0, j0),
            kcs[pr][:, d0 * KK + j0:d0 * KK + j0 + 1])
        p1s[(pr, b)] = p1

    blkq = []

    def blk_muls(pr, b):
        kc = kcs[pr]
        p2 = e.parts.tile([128, BLK, W], bf16, tag="p2", bufs=4,
                          name=f"p2_{pr}{b}")
        p3 = e.parts.tile([128, BLK, W], bf16, tag="p3", name=f"p3_{pr}{b}")
        p4 = e.parts.tile([128, BLK, W], bf16, tag="p4", name=f"p4_{pr}{b}")
        (d1, j1), (d2, j2), (d3, j3) = DVE_TAPS[1:]
        nc.vector.tensor_scalar_mul(
            p2[:], xv(pr, b, d1, j1), kc[:, d1 * KK + j1:d1 * KK + j1 + 1])
        nc.vector.tensor_scalar_mul(
            p3[:], xv(pr, b, d2, j2), kc[:, d2 * KK + j2:d2 * KK + j2 + 1])
        # the P2+=P3 merge: its operands are ready earliest and its
        # result is needed last, so for odd blocks it runs on the Pool
        # engine (tensor_tensor add) to shed DVE's 2x-mode add pressure
        if DMA_MERGE and variant != 7:
            nc.gpsimd.dma_start(out=p2[:], in_=p3[:], accum_op=ALU.add)
        elif b % 2 == 1:
            nc.gpsimd.tensor_tensor(p2[:], p2[:], p3[:], ALU.add)
        else:
            nc.vector.tensor_add(p2[:], p2[:], p3[:])
        q = None
        if variant != 8:
            q = e.parts.tile([128, BLK, W], bf16, tag="q", name=f"q_{pr}{b}")
            nc.gpsimd.tensor_scalar_mul(
                q[:], xv(pr, b, *POOL_TAP), kcs[pr][:, t_pool:t_pool + 1])
        nc.vector.tensor_scalar_mul(
            p4[:], xv(pr, b, d3, j3), kc[:, d3 * KK + j3:d3 * KK + j3 + 1])
        return (p2, p4, q)

    def blk_adds(pr, b, tiles):
        p2, p4, q = tiles
        nc.vector.tensor_add(p2[:], p2[:], p4[:])
        nc.vector.tensor_add(p2[:], p2[:], p1s[(pr, b)][:])
        if q is not None:
            nc.vector.tensor_add(p2[:], p2[:], q[:])
        part_of[(pr, b)] = p2

    for pr, b in blks:
        blkq.append((pr, b, blk_muls(pr, b)))
        if len(blkq) > 1:
            qpr, qb, qt = blkq.pop(0)
            blk_adds(qpr, qb, qt)
    while blkq:
        qpr, qb, qt = blkq.pop(0)
        blk_adds(qpr, qb, qt)

    for pr in range(PAIRS):
        xt = xts[pr]
        # PE groups: taps + injection -> lrelu -> conv -> bias -> store
        for g in range(NGRP):
            i0 = g * RPG
            taps = list(PE_TAPS)
            allpe = first and pr == 0 and g < BLK // RPG
            if allpe:
                taps += [POOL_TAP] + DVE_TAPS
            pa = e.psA.tile([128, RPG * W], f32, tag="pa", name=f"pa{g}")
            HB = RPG // 2  # rows per 512-col sub-chunk (one PSUM bank)
            NW = HB * W
            for t_idx, (di, dj) in enumerate(taps):
                dg = bset.diag[pr][(di, dj)]
                last = ((variant in (5, 6)) or allpe) \
                    and t_idx == len(taps) - 1
                for c2 in range(2):
                    j0 = i0 + c2 * HB
                    nc.tensor.matmul(
                        pa[:, c2 * NW:(c2 + 1) * NW],
                        lhsT=dg[:],
                        rhs=xt[:, j0 + di:j0 + di + HB, dj:dj + W],
                        start=(t_idx == 0), stop=last,
                        skip_group_check=True)
            if variant not in (5, 6) and not allpe:
                part = part_of[(pr, i0 // BLK)]
                roff = i0 % BLK
                for c2 in range(2):
                    nc.tensor.matmul(
                        pa[:, c2 * NW:(c2 + 1) * NW], lhsT=e.eyebf_sb[:],
                        rhs=part[:, roff + c2 * HB:roff + c2 * HB + HB, :],
                        start=False, stop=True, skip_group_check=True)

            yt = e.ys.tile([128, RPG * W], bf16, tag="yt")
            nc.scalar.activation(yt[:], pa[:], AF.Prelu, alpha=0.1)

            # conv/bias/store run one group behind so the PE never waits
            # on the Prelu round-trip (taps of group g+1 fill the gap)
            pend.append((yt, pr, i0))
            if len(pend) > 1:
                _conv_stage(e, nc, AF, f32, bf16, pend.pop(0), NW, variant)
            if steps and (pr * NGRP + g) >= 2:
                steps.pop(0)()
    while pend:
        item = pend.pop(0)
        _conv_stage(e, nc, AF, f32, bf16, item, NW, variant,
                    final=(len(pend) == 0))
    while steps:
        steps.pop(0)()


def _conv_stage(e, nc, AF, f32, bf16, item, NW, variant, final=False):
    """Conv + bias for one group; output rides a 2-group [128, 2*RPG*W]
    tile so each store DMA covers 16 rows (halves HWDGE/queue pressure).
    The final group is drained in 512-col slivers so the last Act pass and
    store overlap instead of serializing into a ~5us tail."""
    yt, pr, i0 = item
    if variant == 4:
        nc.sync.dma_start(
            out=e.ov[pr, :, i0:i0 + RPG, :],
            in_=yt[:].rearrange("p (r w) -> p r w", r=RPG))
        return
    pb = e.psB.tile([128, RPG * W], f32, tag="pb")
    for c2 in range(2):
        nc.tensor.matmul(pb[:, c2 * NW:(c2 + 1) * NW],
                         lhsT=e.convt_sb[:],
                         rhs=yt[:, c2 * NW:(c2 + 1) * NW],
                         start=True, stop=True)
    half = (i0 // RPG) % 2
    if half == 0:
        e._ot2 = e.osb.tile([128, 2, RPG * W], bf16, tag="ot")
    ot2 = e._ot2
    if final:
        HB = RPG // 2
        for c2 in range(2):
            nc.scalar.activation(ot2[:, half, c2 * NW:(c2 + 1) * NW],
                                 pb[:, c2 * NW:(c2 + 1) * NW],
                                 AF.Identity, bias=e.bcol_sb[:, 0:1])
            if variant != 2:
                nc.sync.dma_start(
                    out=e.ov[pr, :, i0 + c2 * HB:i0 + (c2 + 1) * HB, :],
                    in_=ot2[:, half, c2 * NW:(c2 + 1) * NW].rearrange(
                        "p (r w) -> p r w", r=HB))
        if half == 1 and variant != 2:
            nc.sync.dma_start(
                out=e.ov[pr, :, i0 - RPG:i0, :],
                in_=ot2[:, 0, :].rearrange("p (r w) -> p r w", r=RPG))
        return
    nc.scalar.activation(ot2[:, half, :], pb[:], AF.Identity,
                         bias=e.bcol_sb[:, 0:1])
    if half == 1 and variant != 2:
        nc.sync.dma_start(
            out=e.ov[pr, :, i0 - RPG:i0 + RPG, :],
            in_=ot2[:].rearrange("p h (r w) -> p (h r) w", r=RPG))


def get_nc(repeat=1, n_taps=9, unroll=None):
    key = ("nc", repeat, n_taps, unroll)
    if key not in _CACHE:
        _CACHE[key] = _build(repeat, n_taps, unroll)
    return _CACHE[key]


def make_in_maps(x0, v, ca_w1, ca_w2, k_w1, k_w2, conv_w, conv_b):
    bf = ml_dtypes.bfloat16
    w1cat = np.concatenate(
        [np.asarray(k_w1).T, np.asarray(ca_w1).T], axis=1
    ).astype(bf)
    w1cat = np.ascontiguousarray(w1cat)
    caw2t = np.ascontiguousarray(
        np.concatenate([ca_w2.T, ca_w2.T], axis=1)).astype(bf)
    kw2t = np.ascontiguousarray(k_w2.T).astype(bf)
    convt = np.zeros((128, 128), dtype=bf)
    cwt = conv_w.T.astype(bf)
    convt[0:64, 0:64] = cwt
    convt[64:128, 64:128] = cwt
    bcol = np.tile(conv_b.astype(np.float32), 2)[:, None].copy()
    eye = np.eye(128, dtype=np.float32)
    eyebf = np.eye(128, dtype=bf)
    xpad = np.zeros((B, C, H, WP), dtype=bf)
    xpad[:, :, :, 1:1 + W] = np.asarray(x0)
    in_maps = []
    for k in range(N_CORES):
        sl = slice(k * BPC, (k + 1) * BPC)
        in_maps.append({
            "x": np.ascontiguousarray(xpad[sl]),
            "vt": np.ascontiguousarray(v[sl].T).astype(bf),
            "w1cat": w1cat, "caw2t": caw2t, "kw2t": kw2t,
            "convt": convt, "bcol": bcol, "eye": eye, "eyebf": eyebf,
        })
    return in_maps


def kernel(x0, v, ca_w1, ca_w2, k_w1, k_w2, conv_w, conv_b):
    from concourse.bass_utils import run_bass_kernel_spmd

    nc = get_nc()
    in_maps = make_in_maps(x0, v, ca_w1, ca_w2, k_w1, k_w2, conv_w, conv_b)
    res = run_bass_kernel_spmd(nc, in_maps, list(range(N_CORES)))
    return np.concatenate([res.results[i]["out"] for i in range(N_CORES)],
                          axis=0).astype(np.float32)


# revision 46
# speedup vs baseline: 12.3283x; 12.3283x over previous
"""Trainium2 Bass kernel for the per-sample dynamic-depthwise-conv block.

Computation (per sample b):
    att  = sigmoid(lrelu(v @ ca_w1.T) @ ca_w2.T)            # [b, 64]
    kern = (lrelu(v @ k_w1.T) @ k_w2.T).reshape(b*64,1,3,3) # per-(b,c) 3x3
    y    = lrelu(depthwise3x3(x0 * att, kern))
    out  = conv1x1(y, conv_w) + conv_b

Strategy: data-parallel over batch across 8 cores (4 samples/core).  On each
core, samples are processed in 2 "pairs"; a pair's 2x64 channels fill the 128
SBUF partitions.  The attention gate is folded into the generated tap weights
(dw(att*x) == att*dw(x) per channel).  x is pre-padded on the host to width
130 (zero side columns) so each input DMA moves full 4160-byte contiguous
row blocks; the vertical one-row borders are zeroed in SBUF.

The 9 depthwise taps are spread over four engines plus the DMA engines so no
single engine owns more than ~70% of the span:
  - PE: the 4 odd-column taps (0,1),(1,1),(2,1),(1,2) as PSUM-accumulated
    matmuls with diagonal bf16 lhsT against shifted views of the padded
    tile, plus one identity-matmul injection of the SBUF partial per 8-row
    group, plus the final 1x1 conv.
  - DVE: the 4 even-column taps (0,0),(0,2),(1,0),(2,2) as 4x-mode
    tensor_scalar products, merged with two 2x-mode adds.
  - GpSimd(Pool): tap (2,0) fused product+merge into the first DVE product
    via scalar_tensor_tensor.
  - DMA (gpsimd SWDGE): one in-place accumulate merge (P2 += P3) rides the
    otherwise ~60%-idle DMA engines.
Leaky-relu runs on the Scalar engine (Prelu, PSUM->SBUF, bf16) over
[128,1024] groups; the final 1x1 conv is one matmul per 8-row group with a
block-diagonal [conv_w.T, conv_w.T] lhsT; conv bias rides the Scalar
engine's Identity activation during the PSUM->SBUF copy; outputs leave as
bf16 and are widened to fp32 on the host.

Queue layout: x row-block loads on the SP HWDGE queue (emitted first so
nothing delays them), constants + output stores on the Act HWDGE queue,
gather DMAs on SP after the loads, the merge-accumulate on the Pool SWDGE
queue.  The tiny MLP runs as one fused [64,72] matmul (k_w1|ca_w1) + Prelu,
then sigmoid/kern stages; per-pair tap-scalar prep is software-pipelined one
iteration ahead (ping-pong A/B buffer sets) so the MLP->gather->diag chain
never sits on the PE critical path at the loop seam.
"""

import sys

if "/opt/trn_rl_repo" not in sys.path:
    sys.path.append("/opt/trn_rl_repo")

import numpy as np
import ml_dtypes

B, C, H, W = 32, 64, 128, 128
KK = 3
RED = 8
N_CORES = 8
BPC = B // N_CORES          # samples per core (4)
PAIRS = BPC // 2            # sample pairs per core (2)
HP, WP = H + 2, W + 2       # padded image dims (130); width padded on host
RPG = 8                     # output rows per PE group -> N = 8*128 = 1024
NGRP = H // RPG             # 16 groups per pair
BLK = 32                    # DVE partial block rows
NBLK = H // BLK             # 4 blocks per pair

# tap assignment (di, dj); t = di*3 + dj
PE_TAPS = [(0, 1), (1, 1), (2, 1), (1, 2), (2, 0)]   # PE diag matmuls
DVE_TAPS = [(0, 0), (0, 2), (1, 0), (2, 2)]   # even dj -> DVE 4x muls
POOL_TAP = (2, 0)                              # tap kept OFF gpsimd: real
# Q7 tensor ops run ~10x below the cost-model rate (576us/iter measured
# with pool products+adds vs ~70us without); Pool only does DMA work
import os as _os
DMA_MERGE = _os.environ.get("K_DMA_MERGE", "0") == "1"   # SWDGE accum-DMA merge: compiles + small-probe passes, but the full kernel NEFF dies at runtime with it on
GATHER_MERGED = _os.environ.get("K_GATHER_MERGED", "1") == "1"
GATHER_POOL = _os.environ.get("K_GATHER_POOL", "1") == "1"
PAIR1_POOL = _os.environ.get("K_PAIR1_POOL", "1") == "1"
XSPLIT = 8                                     # x row-block DMAs per pair

_CACHE = {}


class _Env:
    pass


def _build(repeat=1, n_taps=9, unroll=None):
    import concourse.bass as bass  # noqa: F401
    from concourse import bacc, tile, mybir

    f32 = mybir.dt.float32
    bf16 = mybir.dt.bfloat16

    e = _Env()
    e.mybir = mybir
    e.AF = mybir.ActivationFunctionType
    e.ALU = mybir.AluOpType
    e.f32 = f32
    e.bf16 = bf16

    nc = bacc.Bacc(None, target_bir_lowering=False, debug=False)
    e.nc = nc

    e.x = nc.dram_tensor("x", [BPC, C, H, WP], bf16, kind="ExternalInput")
    e.vt = nc.dram_tensor("vt", [C, BPC], bf16, kind="ExternalInput")
    e.w1cat = nc.dram_tensor("w1cat", [C, C + RED], bf16,
                             kind="ExternalInput")
    e.caw2t = nc.dram_tensor("caw2t", [RED, 2 * C], bf16,
                             kind="ExternalInput")
    e.kw2t = nc.dram_tensor("kw2t", [C, C * KK * KK], bf16,
                            kind="ExternalInput")
    e.convt = nc.dram_tensor("convt", [128, 128], bf16, kind="ExternalInput")
    e.bcol = nc.dram_tensor("bcol", [128, 1], f32, kind="ExternalInput")
    e.eye = nc.dram_tensor("eye", [128, 128], f32, kind="ExternalInput")
    e.eyebf = nc.dram_tensor("eyebf", [128, 128], bf16, kind="ExternalInput")
    e.out = nc.dram_tensor("out", [BPC, C, H, W], bf16, kind="ExternalOutput")

    with tile.TileContext(nc) as tc:
        with (
            tc.tile_pool(name="consts", bufs=1) as consts,
            tc.tile_pool(name="stage", bufs=1) as stage,
            tc.tile_pool(name="diags", bufs=1) as diags,
            tc.tile_pool(name="xin", bufs=2) as xin,
            tc.tile_pool(name="parts", bufs=2) as parts,
            tc.tile_pool(name="ys", bufs=3) as ys,
            tc.tile_pool(name="os", bufs=3) as osb,
            tc.tile_pool(name="psA", bufs=2, space="PSUM") as psA,
            tc.tile_pool(name="psB", bufs=2, space="PSUM") as psB,
        ):
            e.consts, e.stage, e.diags = consts, stage, diags
            e.xin, e.parts, e.ys, e.osb = xin, parts, ys, osb
            e.psA, e.psB = psA, psB

            # ---- constants into SBUF (Act HWDGE queue; keeps SP free for
            # the x row-block loads that gate first compute) ----
            e.vt_sb = consts.tile([C, BPC], bf16)
            e.w1cat_sb = consts.tile([C, C + RED], bf16)
            e.caw2t_sb = consts.tile([C + RED, 2 * C], bf16)  # rows 64:72
            e.kw2t_sb = consts.tile([C, C * KK * KK], bf16)
            e.scratch = consts.tile([128, 512], bf16)     # PE warmup fodder
            e.convt_sb = consts.tile([128, 128], bf16)
            e.bcol_sb = consts.tile([128, 1], f32)
            e.eye_sb = consts.tile([128, 128], f32)
            e.eyebf_sb = consts.tile([128, 128], bf16)
            # ordered by when the prep chain needs them: MLP weights first,
            # then eye (diag builds), then the conv-stage constants.  On the
            # SP queue *before* the x row-block loads: the x transfers are
            # 1.5us each on the shared DMA FIFO and would starve these tiny
            # loads for ~10us if queued first.
            for t, d in (
                (e.vt_sb[:], e.vt), (e.w1cat_sb[:], e.w1cat),
                (e.caw2t_sb[C:C + RED, :], e.caw2t),
                (e.eye_sb[:], e.eye), (e.kw2t_sb[:], e.kw2t),
                (e.eyebf_sb[:], e.eyebf), (e.convt_sb[:], e.convt),
                (e.bcol_sb[:], e.bcol),
            ):
                nc.sync.dma_start(out=t, in_=d.ap())

            e.xv = e.x.ap().rearrange("(pr s) c h w -> pr (s c) h w",
                                      pr=PAIRS)
            e.ov = e.out.ap().rearrange("(pr s) c h w -> pr (s c) h w",
                                        pr=PAIRS)

            setA = _alloc_bufset(e, "A")
            setB = _alloc_bufset(e, "B")

            # PE p-state warmup: junk matmuls on an unwritten scratch tile
            # keep the PE continuously busy from t~0.5us so it reaches full
            # clock (2.4 GHz needs 3us of continuous execution) before the
            # real tap stream begins.  Interleaved through the initial prep
            # so no real instruction is delayed by more than one 512-col
            # matmul.
            nc.gpsimd.memset(e.scratch[:], 0.0)

            def warm():
                psw = e.psA.tile([128, 512], f32, tag="pa", name="wm")
                nc.tensor.matmul(psw[:], lhsT=e.scratch[:, 0:128],
                                 rhs=e.scratch[:], start=True, stop=True,
                                 skip_group_check=True)

            for _ in range(4):
                warm()
            xts0 = _load_x0(e, n_taps)
            _prep(e, setA, warm=warm)
            if repeat == 1:
                n_un = unroll or 1
                sets = [setA, setB]
                for u in range(n_un):
                    stp = (_prep_steps(e, sets[(u + 1) % 2])
                           if u + 1 < n_un else None)
                    _main(e, sets[u % 2], stp, variant=n_taps,
                          first=(u == 0), xts=(xts0 if u == 0 else None))
            else:
                assert unroll is None
                half, odd = divmod(repeat, 2)
                if half > 0:
                    with tc.For_i(0, half, 1):
                        _main(e, setA, _prep_steps(e, setB), variant=n_taps)
                        _main(e, setB, _prep_steps(e, setA), variant=n_taps)
                if odd:
                    _main(e, setA, variant=n_taps)

    nc.compile()
    return nc


def _alloc_bufset(e, nm):
    """Per-iteration tap-scalar buffers: diag tiles for the PE taps and the
    d-column tile per pair."""
    s = _Env()
    s.diag = [{} for _ in range(PAIRS)]
    s.dcols = []
    for pr in range(PAIRS):
        s.dcols.append(
            e.stage.tile([128, KK * KK], e.f32, tag=f"d{nm}{pr}",
                         name=f"d{nm}{pr}"))
        # pair0 carries diags for all 9 taps: the kernel-start block runs
        # entirely on PE (no partial-injection dependency)
        tl = PE_TAPS + (DVE_TAPS if pr == 0 else [])
        for (di, dj) in tl:
            t = di * KK + dj
            s.diag[pr][(di, dj)] = e.diags.tile(
                [128, 128], e.bf16, tag=f"diag{nm}{pr}_{t}",
                name=f"diag{nm}{pr}_{t}")
    return s


def _prep_steps(e, bset, first=False):
    """MLP + per-pair tap-scalar prep as a list of emit-closures so the
    caller can interleave them between main-loop groups (hides the
    matmul->activation round-trip latencies behind main-loop work).
    `first` additionally builds pair0's pool/DVE-tap diags for the all-PE
    kernel-start block."""
    nc, AF, f32 = e.nc, e.AF, e.f32
    NK = C * KK * KK
    st = _Env()
    steps = []

    def s_h():
        # fused hidden: h[0:64] = lrelu(k_w1 @ v), h[64:72] = lrelu(ca_w1 @ v)
        ps_h = e.psA.tile([C + RED, BPC], f32, tag="pa", name="ps_h")
        nc.tensor.matmul(ps_h[:], lhsT=e.w1cat_sb[:], rhs=e.vt_sb[:],
                         start=True, stop=True)
        st.h = e.stage.tile([C + RED, BPC], e.bf16, tag="h", name="h")
        nc.scalar.activation(st.h[:], ps_h[:], AF.Prelu, alpha=0.1)

    def s_att():
        # lhsT = [caw2t | caw2t] so the attention lands duplicated across
        # both 64-partition halves: att2[p, b] = att[p % 64, b].  The
        # per-pair attpp columns then come from cheap same-partition Act
        # copies instead of partition-scatter DMAs.
        ps_att = e.psA.tile([2 * C, BPC], f32, tag="pa", name="ps_att")
        nc.tensor.matmul(ps_att[:], lhsT=e.caw2t_sb[C:C + RED, :],
                         rhs=st.h[C:C + RED, :], start=True, stop=True)
        st.att = e.stage.tile([2 * C, BPC], f32, tag="att", name="att")
        nc.scalar.activation(st.att[:], ps_att[:], AF.Sigmoid)

    def s_kern():
        ps_k = e.psA.tile([BPC, NK], f32, tag="pa", name="ps_k")
        nc.tensor.matmul(ps_k[:, 0:512], lhsT=st.h[0:C, :],
                         rhs=e.kw2t_sb[:, 0:512], start=True, stop=True)
        nc.tensor.matmul(ps_k[:, 512:NK], lhsT=st.h[0:C, :],
                         rhs=e.kw2t_sb[:, 512:NK], start=True, stop=True)
        st.kern = e.stage.tile([BPC, NK], f32, tag="kern", name="kern")
        nc.scalar.activation(st.kern[:], ps_k[:], AF.Copy)

    def s_gather():
        # partition-scatter gathers ride the Pool SWDGE queue: its SEQ
        # dispatch is ~25ns vs the ~650ns HWDGE config that would stall
        # the SP/Act queues mid-prep
        st.dtap = e.stage.tile([128, PAIRS, KK * KK], f32, tag="dtap",
                               name="dtap")
        st.attpp = e.stage.tile([128, PAIRS], f32, tag="attpp",
                                name="attpp")
        dma = nc.gpsimd.dma_start if GATHER_POOL else nc.sync.dma_start
        for pr in range(PAIRS):
            if GATHER_MERGED:
                src = st.kern[2 * pr:2 * pr + 2, :].rearrange(
                    "s (c t) -> s c t", c=C)
                dma(out=st.dtap[:, pr:pr + 1, :], in_=src)
            else:
                for sdx in range(2):
                    b = pr * 2 + sdx
                    src = st.kern[b:b + 1, :].rearrange(
                        "o (c t) -> o c t", c=C)
                    dma(out=st.dtap[C * sdx:C * (sdx + 1), pr:pr + 1, :],
                        in_=src)
            for sdx in range(2):
                b = pr * 2 + sdx
                nc.scalar.activation(
                    st.attpp[C * sdx:C * (sdx + 1), pr:pr + 1],
                    st.att[C * sdx:C * (sdx + 1), b:b + 1], AF.Copy)

    def s_dcol(pr):
        def f():
            nc.vector.tensor_scalar_mul(
                bset.dcols[pr][:], st.dtap[:, pr, :],
                st.attpp[:, pr:pr + 1])
        return f

    def s_diag(pr, di, dj):
        def f():
            t = di * KK + dj
            nc.scalar.activation(
                bset.diag[pr][(di, dj)][:], e.eye_sb[:], AF.Copy,
                scale=bset.dcols[pr][:, t:t + 1])
        return f

    steps = [s_h, s_att, s_kern, s_gather]
    for pr in range(PAIRS):
        steps.append(s_dcol(pr))
        tl = list(PE_TAPS)
        if first and pr == 0:
            tl += DVE_TAPS
        for (di, dj) in tl:
            steps.append(s_diag(pr, di, dj))
    return steps


def _prep(e, bset, warm=None):
    for i, f in enumerate(_prep_steps(e, bset, first=True)):
        f()
        if warm is not None and i < 22:
            warm()


def _load_x0(e, variant):
    """Allocate both pairs' padded tiles, zero the borders, and start
    pair0's row-block loads on the SP HWDGE queue.  pair1's loads are
    emitted later (in _main) on the Pool SWDGE queue so their transfers
    enter the shared DMA-engine FIFO *after* the prep gathers — otherwise
    the tiny gather scatters queue behind ~24us of x traffic."""
    nc, bf16 = e.nc, e.bf16
    xts = []
    rstep = H // XSPLIT
    for pr in range(PAIRS):
        xt = e.xin.tile([128, HP, WP], bf16, tag="xt", name=f"xt{pr}")
        xts.append(xt)
        # zero the top/bottom border rows (side columns are zero from the
        # host-side width padding)
        nc.gpsimd.memset(xt[:, 0, :], 0.0)
        nc.gpsimd.memset(xt[:, HP - 1, :], 0.0)
    for k in (range(XSPLIT) if variant != 1 else []):
        r0 = k * rstep
        nc.sync.dma_start(
            out=xts[0][:, 1 + r0:1 + r0 + rstep, :],
            in_=e.xv[0, :, r0:r0 + rstep, :])
    return xts


def _main(e, bset, steps=None, variant=9, first=False, xts=None):
    """Two-pair main loop reading tap scalars from `bset`.  `steps` are
    next-iteration prep emitters, drained one per PE group."""
    nc, AF, ALU, f32, bf16 = e.nc, e.AF, e.ALU, e.f32, e.bf16
    steps = list(steps) if steps else []
    pend = []
    NW = (RPG // 2) * W

    t_pool = POOL_TAP[0] * KK + POOL_TAP[1]

    if xts is None:
        xts = _load_x0(e, variant)
    rstep = H // XSPLIT
    for pr in range(1, PAIRS):
        dma = nc.gpsimd.dma_start if PAIR1_POOL else nc.sync.dma_start
        for k in (range(XSPLIT) if variant != 1 else []):
            r0 = k * rstep
            dma(out=xts[pr][:, 1 + r0:1 + r0 + rstep, :],
                in_=e.xv[pr, :, r0:r0 + rstep, :])

    # non-PE partial per BLK-row block:
    #   P1 = x(0,0)*k00            (DVE 4x)
    #   Q  = x(2,0)*k20            (Pool tensor_scalar product)
    #   P2 = x(0,2)*k02            (DVE 4x)
    #   P3 = x(1,0)*k10            (DVE 4x)
    #   P2 += P3                   (SWDGE dma accum or DVE add)
    #   P4 = x(2,2)*k22            (DVE 4x)
    #   P2 += P4; P2 += Q; P2 += P1   (DVE adds)
    # (scalar_tensor_tensor is not in the Pool engine's ISA, so the (2,0)
    # tap is a plain product folded by DVE.)  All P1 products are emitted
    # first, then per block the remaining muls with the adds software-
    # pipelined one block behind so the in-order DVE stream never stalls on
    # the Pool/DMA merge latency.  When `first` (kernel start), pair0/block0
    # runs entirely on PE diag matmuls instead — the first injects would
    # otherwise wait on the whole DVE/Pool chain.
    part_of = {}
    kcs = bset.dcols

    def xv(pr, b, di, dj):
        r0 = b * BLK
        return xts[pr][:, r0 + di:r0 + di + BLK, dj:dj + W]

    blks = [(pr, b) for pr in range(PAIRS)
            for b in ([] if variant == 5 else range(NBLK))
            if not (first and pr == 0 and b == 0)]

    p1s = {}
    for pr, b in blks:
        (d0, j0) = DVE_TAPS[0]
        p1 = e.parts.tile([128, BLK, W], bf16, tag="p1", bufs=3,
                          name=f"p1_{pr}{b}")
        nc.vector.tensor_scalar_mul(
            p1[:], xv(pr, b, d0, j0),
            kcs[pr][:, d0 * KK + j0:d0 * KK + j0 + 1])
        p1s[(pr, b)] = p1

    blkq = []

    def blk_muls(pr, b):
        kc = kcs[pr]
        p2 = e.parts.tile([128, BLK, W], bf16, tag="p2", bufs=4,
                          name=f"p2_{pr}{b}")
        p3 = e.parts.tile([128, BLK, W], bf16, tag="p3", name=f"p3_{pr}{b}")
        p4 = e.parts.tile([128, BLK, W], bf16, tag="p4", name=f"p4_{pr}{b}")
        (d1, j1), (d2, j2), (d3, j3) = DVE_TAPS[1:]
        nc.vector.tensor_scalar_mul(
            p2[:], xv(pr, b, d1, j1), kc[:, d1 * KK + j1:d1 * KK + j1 + 1])
        nc.vector.tensor_scalar_mul(
            p3[:], xv(pr, b, d2, j2), kc[:, d2 * KK + j2:d2 * KK + j2 + 1])
        if DMA_MERGE and variant != 7:
            nc.gpsimd.dma_start(out=p2[:], in_=p3[:], accum_op=ALU.add)
        else:
            nc.vector.tensor_add(p2[:], p2[:], p3[:])
        nc.vector.tensor_scalar_mul(
            p4[:], xv(pr, b, d3, j3), kc[:, d3 * KK + j3:d3 * KK + j3 + 1])
        return (p2, p4, None)

    def blk_adds(pr, b, tiles):
        p2, p4, q = tiles
        nc.vector.tensor_add(p2[:], p2[:], p4[:])
        nc.vector.tensor_add(p2[:], p2[:], p1s[(pr, b)][:])
        if q is not None:
            nc.vector.tensor_add(p2[:], p2[:], q[:])
        part_of[(pr, b)] = p2

    for pr, b in blks:
        blkq.append((pr, b, blk_muls(pr, b)))
        if len(blkq) > 1:
            qpr, qb, qt = blkq.pop(0)
            blk_adds(qpr, qb, qt)
    while blkq:
        qpr, qb, qt = blkq.pop(0)
        blk_adds(qpr, qb, qt)

    for pr in range(PAIRS):
        xt = xts[pr]
        # PE groups: taps + injection -> lrelu -> conv -> bias -> store
        for g in range(NGRP):
            i0 = g * RPG
            taps = list(PE_TAPS)
            allpe = first and pr == 0 and g < BLK // RPG
            if allpe:
                taps += DVE_TAPS
            pa = e.psA.tile([128, RPG * W], f32, tag="pa", name=f"pa{g}")
            HB = RPG // 2  # rows per 512-col sub-chunk (one PSUM bank)
            NW = HB * W
            for t_idx, (di, dj) in enumerate(taps):
                dg = bset.diag[pr][(di, dj)]
                last = ((variant in (5, 6)) or allpe) \
                    and t_idx == len(taps) - 1
                for c2 in range(2):
                    j0 = i0 + c2 * HB
                    nc.tensor.matmul(
                        pa[:, c2 * NW:(c2 + 1) * NW],
                        lhsT=dg[:],
                        rhs=xt[:, j0 + di:j0 + di + HB, dj:dj + W],
                        start=(t_idx == 0), stop=last,
                        skip_group_check=True)
            if variant not in (5, 6) and not allpe:
                part = part_of[(pr, i0 // BLK)]
                roff = i0 % BLK
                for c2 in range(2):
                    nc.tensor.matmul(
                        pa[:, c2 * NW:(c2 + 1) * NW], lhsT=e.eyebf_sb[:],
                        rhs=part[:, roff + c2 * HB:roff + c2 * HB + HB, :],
                        start=False, stop=True, skip_group_check=True)

            yt = e.ys.tile([128, RPG * W], bf16, tag="yt")
            nc.scalar.activation(yt[:], pa[:], AF.Prelu, alpha=0.1)

            # conv/bias/store run one group behind so the PE never waits
            # on the Prelu round-trip (taps of group g+1 fill the gap)
            pend.append((yt, pr, i0))
            if len(pend) > 1:
                _conv_stage(e, nc, AF, f32, bf16, pend.pop(0), NW, variant)
            if steps and (pr * NGRP + g) >= 2:
                steps.pop(0)()
    while pend:
        item = pend.pop(0)
        _conv_stage(e, nc, AF, f32, bf16, item, NW, variant,
                    final=(len(pend) == 0))
    while steps:
        steps.pop(0)()


def _conv_stage(e, nc, AF, f32, bf16, item, NW, variant, final=False):
    """Conv + bias for one group; output rides a 2-group [128, 2*RPG*W]
    tile so each store DMA covers 16 rows (halves HWDGE/queue pressure).
    The final group is drained in 512-col slivers so the last Act pass and
    store overlap instead of serializing into a ~5us tail."""
    yt, pr, i0 = item
    if variant == 4:
        nc.sync.dma_start(
            out=e.ov[pr, :, i0:i0 + RPG, :],
            in_=yt[:].rearrange("p (r w) -> p r w", r=RPG))
        return
    pb = e.psB.tile([128, RPG * W], f32, tag="pb")
    for c2 in range(2):
        nc.tensor.matmul(pb[:, c2 * NW:(c2 + 1) * NW],
                         lhsT=e.convt_sb[:],
                         rhs=yt[:, c2 * NW:(c2 + 1) * NW],
                         start=True, stop=True)
    half = (i0 // RPG) % 2
    if half == 0:
        e._ot2 = e.osb.tile([128, 2, RPG * W], bf16, tag="ot")
    ot2 = e._ot2
    if final:
        HB = RPG // 2
        for c2 in range(2):
            nc.scalar.activation(ot2[:, half, c2 * NW:(c2 + 1) * NW],
                                 pb[:, c2 * NW:(c2 + 1) * NW],
                                 AF.Identity, bias=e.bcol_sb[:, 0:1])
            if variant != 2:
                nc.sync.dma_start(
                    out=e.ov[pr, :, i0 + c2 * HB:i0 + (c2 + 1) * HB, :],
                    in_=ot2[:, half, c2 * NW:(c2 + 1) * NW].rearrange(
                        "p (r w) -> p r w", r=HB))
        if half == 1 and variant != 2:
            nc.sync.dma_start(
                out=e.ov[pr, :, i0 - RPG:i0, :],
                in_=ot2[:, 0, :].rearrange("p (r w) -> p r w", r=RPG))
        return
    nc.scalar.activation(ot2[:, half, :], pb[:], AF.Identity,
                         bias=e.bcol_sb[:, 0:1])
    if half == 1 and variant != 2:
        nc.sync.dma_start(
            out=e.ov[pr, :, i0 - RPG:i0 + RPG, :],
            in_=ot2[:].rearrange("p h (r w) -> p (h r) w", r=RPG))


def get_nc(repeat=1, n_taps=9, unroll=None):
    key = ("nc", repeat, n_taps, unroll)
    if key not in _CACHE:
        _CACHE[key] = _build(repeat, n_taps, unroll)
    return _CACHE[key]


def make_in_maps(x0, v, ca_w1, ca_w2, k_w1, k_w2, conv_w, conv_b):
    bf = ml_dtypes.bfloat16
    w1cat = np.concatenate(
        [np.asarray(k_w1).T, np.asarray(ca_w1).T], axis=1
    ).astype(bf)
    w1cat = np.ascontiguousarray(w1cat)
    caw2t = np.ascontiguousarray(
        np.concatenate([ca_w2.T, ca_w2.T], axis=1)).astype(bf)
    kw2t = np.ascontiguousarray(k_w2.T).astype(bf)
    convt = np.zeros((128, 128), dtype=bf)
    cwt = conv_w.T.astype(bf)
    convt[0:64, 0:64] = cwt
    convt[64:128, 64:128] = cwt
    bcol = np.tile(conv_b.astype(np.float32), 2)[:, None].copy()
    eye = np.eye(128, dtype=np.float32)
    eyebf = np.eye(128, dtype=bf)
    xpad = np.zeros((B, C, H, WP), dtype=bf)
    xpad[:, :, :, 1:1 + W] = np.asarray(x0)
    in_maps = []
    for k in range(N_CORES):
        sl = slice(k * BPC, (k + 1) * BPC)
        in_maps.append({
            "x": np.ascontiguousarray(xpad[sl]),
            "vt": np.ascontiguousarray(v[sl].T).astype(bf),
            "w1cat": w1cat, "caw2t": caw2t, "kw2t": kw2t,
            "convt": convt, "bcol": bcol, "eye": eye, "eyebf": eyebf,
        })
    return in_maps


def kernel(x0, v, ca_w1, ca_w2, k_w1, k_w2, conv_w, conv_b):
    from concourse.bass_utils import run_bass_kernel_spmd

    nc = get_nc()
    in_maps = make_in_maps(x0, v, ca_w1, ca_w2, k_w1, k_w2, conv_w, conv_b)
    res = run_bass_kernel_spmd(nc, in_maps, list(range(N_CORES)))
    return np.concatenate([res.results[i]["out"] for i in range(N_CORES)],
                          axis=0).astype(np.float32)
